# revision 1
# baseline (speedup 1.0000x reference)
"""CRF log-likelihood kernel for Trainium2 (Bass/Tile), 8-core data parallel.

out[b] = gold_path_score(b) - logZ(b)

logZ via exp-domain DP with forward and backward chains meeting at t = F:
  fwd:  u_t   = el_t  ⊙ (Wf^T u_{t-1}),      t = 1..F      (u_0 = el_0)
  bwd:  γ_σ   = Wb^T (el_{T+1-σ} ⊙ γ_{σ-1}), σ = 1..T-F    (γ_0 = sink)
Sequences with len <= F finish inside the fwd chain via an absorbing "sink"
label that captures sum_i u_{len-1}[i] exactly at t == len; longer sequences
use the midpoint identity Z = Σ_j α_F[j]·β_F[j], with the bwd chain's sink
"birthing" β = 1 at each sequence's own end time. The two chains are
independent, so PE matmuls of one overlap DVE multiplies of the other.

Layout per core (128 sequences):
  partitions 0..95 = active labels (3 groups x 32), 96..98 = sink row per
  group; psum rows 99..101 = per-group column sums (ones-columns of the
  stationary operand). columns: b_local = 43*g + c.
Scaling: all emissions carry e^{-CSHIFT}; columns are renormalized by their
column sum mid-chain (factor tracked exactly via ACT-Ln of the applied
multiplier). Host adds CSHIFT*len back and picks sink vs combine per length.
Host also does the gold-path gathers (labels/trans only) and final subtract.
"""

import numpy as np
import ml_dtypes

B, T, L = 1024, 512, 32
NCORES = 8
BPC = B // NCORES        # 128 sequences per core
G = 3                    # label groups per core
NCOL = 43                # columns per group (group 2 uses 42 + 1 pad)
NACT = 96                # active label partitions
NPART = 99               # + 3 sink rows
MOUT = 102               # + 3 colsum rows
CSHIFT = 4.5
TEX = T + 1              # el time slices 0..T
TCH = 57                 # el build chunk (9 * 57 = 513)
NCH = TEX // TCH
F = 256                  # fwd ticks; bwd ticks = T - F
SB = T - F
RENORM_EVERY = 128

_prog_cache = {}
last_result = None       # BassKernelResults of the most recent run (for test.py)


def _build_program():
    import concourse.bacc as bacc
    import concourse.tile as tile
    from concourse import mybir

    f32 = mybir.dt.float32
    bf16 = mybir.dt.bfloat16
    AF = mybir.ActivationFunctionType

    nc = bacc.Bacc("TRN2", target_bir_lowering=False, debug=False, num_devices=NCORES)
    lg = nc.dram_tensor("lg", [NACT, TEX, NCOL], f32, kind="ExternalInput")
    el32 = nc.dram_tensor("el32", [G, TEX, NCOL], bf16, kind="ExternalInput")
    wf = nc.dram_tensor("wf", [NPART, MOUT], bf16, kind="ExternalInput")
    wbk = nc.dram_tensor("wbk", [NPART, MOUT], bf16, kind="ExternalInput")
    wbc = nc.dram_tensor("wbc", [2 * G, NPART], f32, kind="ExternalInput")
    wcs = nc.dram_tensor("wcs", [NPART, G], bf16, kind="ExternalInput")
    resf = nc.dram_tensor("resf", [G, NCOL], f32, kind="ExternalOutput")
    resc = nc.dram_tensor("resc", [G, NCOL], f32, kind="ExternalOutput")

    with tile.TileContext(nc) as tc:
        with (
            tc.tile_pool(name="big", bufs=1) as big,
            tc.tile_pool(name="stage", bufs=3) as stage_p,
            tc.tile_pool(name="consts", bufs=1) as consts,
            tc.tile_pool(name="u", bufs=3) as upool,
            tc.tile_pool(name="v", bufs=3) as vpool,
            tc.tile_pool(name="small", bufs=4) as small,
            tc.tile_pool(name="fin", bufs=1) as fin,
            tc.tile_pool(name="psf", bufs=3, space="PSUM") as psfpool,
            tc.tile_pool(name="psb", bufs=3, space="PSUM") as psbpool,
            tc.tile_pool(name="psx", bufs=2, space="PSUM") as psxpool,
        ):
            el_sb = big.tile([NPART, TEX, NCOL], bf16)
            wf_sb = consts.tile([NPART, MOUT], bf16)
            wb_sb = consts.tile([NPART, MOUT], bf16)
            wbc_sb = consts.tile([2 * G, NPART], f32)
            wcs_sb = consts.tile([NPART, G], bf16)
            biasc = consts.tile([128, 1], f32)
            g0 = consts.tile([NPART, NCOL], bf16)
            nc.vector.memset(biasc[:], -CSHIFT)
            nc.vector.memset(g0[:], 0.0)
            nc.vector.memset(g0[NACT:NPART, :], 1.0)

            nc.sync.dma_start(out=wf_sb[:], in_=wf[:])
            nc.sync.dma_start(out=wb_sb[:], in_=wbk[:])
            nc.sync.dma_start(out=wbc_sb[:], in_=wbc[:])
            nc.sync.dma_start(out=wcs_sb[:], in_=wcs[:])
            # sink rows land on partitions 96..98 (one aligned DMA)
            nc.sync.dma_start(out=el_sb[NACT:NPART, :, :], in_=el32[:])
            # active rows: stage raw logits, bulk-exp into el_sb.
            # build order alternates ends: bwd consumes slices from t=T down.
            order = []
            lo, hi = 0, NCH - 1
            while lo <= hi:
                order.append(hi)
                if lo != hi:
                    order.append(lo)
                hi -= 1
                lo += 1
            for ch in order:
                st = stage_p.tile([NACT, TCH, NCOL], f32, tag="stage")
                t0 = ch * TCH
                nc.sync.dma_start(out=st[:], in_=lg[:, t0 : t0 + TCH, :])
                nc.scalar.activation(
                    el_sb[0:NACT, t0 : t0 + TCH, :], st[:], AF.Exp, bias=biasc[0:NACT, :]
                )

            lnrs_f, lnrs_b = [], []
            uprev = el_sb[:, 0, :]
            gprev = g0[:]
            gprev_sbuf = True
            ulast = None
            pb_last = None
            pend_renorm = None
            for k in range(1, max(F, SB) + 1):
                # ---- fwd tick t = k ----
                if k <= F:
                    psf = psfpool.tile([MOUT, NCOL], f32, tag="psf")
                    nc.tensor.matmul(psf[:], wf_sb[:], uprev, start=True, stop=True)
                    un = upool.tile([NPART, NCOL], bf16, tag="u")
                    nc.vector.tensor_mul(un[:], psf[0:NPART, :], el_sb[:, k, :])
                    if k % RENORM_EVERY == 0 and k < F:
                        ts6 = small.tile([2 * G, NCOL], f32, tag="ts6f")
                        nc.vector.tensor_scalar_add(
                            ts6[:], psf[NACT : NACT + 2 * G, :], 1e-30
                        )
                        rr6 = small.tile([2 * G, NCOL], f32, tag="rr6f")
                        nc.vector.reciprocal(rr6[:], ts6[:])
                        psr = psxpool.tile([NPART, NCOL], f32, tag="psr")
                        nc.tensor.matmul(psr[:], wbc_sb[:], rr6[:], start=True, stop=True)
                        un2 = upool.tile([NPART, NCOL], bf16, tag="u2")
                        nc.vector.tensor_mul(un2[:], psr[:], un[:])
                        lnr = fin.tile([G, NCOL], f32, tag=f"lnrf{len(lnrs_f)}")
                        nc.scalar.activation(lnr[:], psr[NACT:NPART, :], AF.Ln)
                        lnrs_f.append(lnr)
                        uprev = un2[:]
                    else:
                        uprev = un[:]
                    if k == F:
                        ulast = uprev
                # ---- bwd tick σ = k, el time T+1-k ----
                if k <= SB:
                    vn = vpool.tile([NPART, NCOL], bf16, tag="v")
                    src = gprev if gprev_sbuf else gprev[0:NPART, :]
                    nc.vector.tensor_mul(vn[:], src, el_sb[:, T + 1 - k, :])
                    if pend_renorm is not None:
                        # apply the deferred renorm factor (can't read two
                        # PSUM operands in one TT)
                        vn2 = vpool.tile([NPART, NCOL], bf16, tag="v2")
                        nc.vector.tensor_mul(vn2[:], pend_renorm[:], vn[:])
                        vn = vn2
                        pend_renorm = None
                    gprev_sbuf = False
                    psb = psbpool.tile([MOUT, NCOL], f32, tag="psb")
                    nc.tensor.matmul(psb[:], wb_sb[:], vn[:], start=True, stop=True)
                    if k % RENORM_EVERY == 0 and k < SB:
                        ts6b = small.tile([2 * G, NCOL], f32, tag="ts6b")
                        nc.vector.tensor_scalar_add(
                            ts6b[:], psb[NACT : NACT + 2 * G, :], 1e-30
                        )
                        rr6b = small.tile([2 * G, NCOL], f32, tag="rr6b")
                        nc.vector.reciprocal(rr6b[:], ts6b[:])
                        psrb = psxpool.tile([NPART, NCOL], f32, tag="psr")
                        nc.tensor.matmul(
                            psrb[:], wbc_sb[:], rr6b[:], start=True, stop=True
                        )
                        pend_renorm = psrb
                        lnrb = fin.tile([G, NCOL], f32, tag=f"lnrb{len(lnrs_b)}")
                        nc.scalar.activation(lnrb[:], psrb[NACT:NPART, :], AF.Ln)
                        lnrs_b.append(lnrb)
                    gprev = psb
                    if k == SB:
                        pb_last = (gprev, gprev_sbuf)

            # ---- combine: w = u_F ⊙ γ_S; Zc = per-group colsum of w ----
            gl, gl_sbuf = pb_last
            wt = vpool.tile([NPART, NCOL], bf16, tag="wt")
            nc.vector.tensor_mul(wt[:], gl if gl_sbuf else gl[0:NPART, :], ulast)
            psc = psxpool.tile([G, NCOL], f32, tag="psr")
            nc.tensor.matmul(psc[:], wcs_sb[:], wt[:], start=True, stop=True)

            # resf = ln(u_F sink) - Σ lnr_f ; resc = ln(Zc) - Σ lnr_f - Σ lnr_b
            accf = fin.tile([G, NCOL], f32, tag="lnu")
            nc.scalar.activation(accf[:], ulast[NACT:NPART, :], AF.Ln)
            for e, lnr in enumerate(lnrs_f):
                nx = fin.tile([G, NCOL], f32, tag=f"fa{e}")
                nc.vector.tensor_sub(nx[:], accf[:], lnr[:])
                accf = nx
            nc.sync.dma_start(out=resf[:], in_=accf[:])

            accc = fin.tile([G, NCOL], f32, tag="lnc")
            nc.scalar.activation(accc[:], psc[:], AF.Ln)
            for e, lnr in enumerate(lnrs_f + lnrs_b):
                nx = fin.tile([G, NCOL], f32, tag=f"ca{e}")
                nc.vector.tensor_sub(nx[:], accc[:], lnr[:])
                accc = nx
            nc.sync.dma_start(out=resc[:], in_=accc[:])

    nc.compile()
    return nc


def _host_prep(logits, trans, labels, seq_lens):
    logits = np.ascontiguousarray(np.asarray(logits), dtype=np.float32)
    trans = np.asarray(trans, dtype=np.float32)
    labels = np.asarray(labels)
    lens = np.clip(np.asarray(seq_lens), 1, T).astype(np.int64)

    # ---- gold path score (host: index gathers over small inputs) ----
    tmask = np.arange(T)[None, :] < lens[:, None]
    unary = np.take_along_axis(logits, labels[..., None].astype(np.int64), axis=2)[..., 0]
    gp = (unary * tmask).sum(1) + (trans[labels[:, :-1], labels[:, 1:]] * tmask[:, 1:]).sum(1)

    # ---- device inputs: mask every t >= len; pad slice t=T = -inf ----
    lgx = logits.copy()
    lgx[~tmask] = -1e9
    lgx = np.concatenate([lgx, np.full((B, 1, L), -1e9, np.float32)], axis=1)

    el32 = (np.arange(TEX)[None, :] >= lens[:, None]).astype(np.float32)  # [B, 513]

    lg_cores, el32_cores = [], []
    for core in range(NCORES):
        b0 = core * BPC
        lgp = np.full((G, 32, TEX, NCOL), -1e9, np.float32)
        e32 = np.zeros((G, TEX, NCOL), np.float32)
        for g in range(G):
            ncols = NCOL if g < 2 else BPC - 2 * NCOL
            bs = b0 + g * NCOL
            lgp[g, :, :, :ncols] = lgx[bs : bs + ncols].transpose(2, 1, 0)
            e32[g, :, :ncols] = el32[bs : bs + ncols].T
            if ncols < NCOL:  # pad column: dummy len==T sequence, active el = 0
                e32[g, T, ncols:] = 1.0
        lg_cores.append(np.ascontiguousarray(lgp).reshape(NACT, TEX, NCOL))
        el32_cores.append(e32.astype(ml_dtypes.bfloat16))

    # ---- stationary operators ----
    E = np.exp(trans).astype(np.float32)
    Wf = np.zeros((NPART, MOUT), np.float32)
    Wb = np.zeros((NPART, MOUT), np.float32)
    Wbc = np.zeros((2 * G, NPART), np.float32)
    Wcs = np.zeros((NPART, G), np.float32)
    for g in range(G):
        a, sk, cs = 32 * g, NACT + g, NPART + g
        Wf[a : a + 32, a : a + 32] = E
        Wf[a : a + 32, sk] = 1.0
        Wf[sk, sk] = 1.0
        Wf[a : a + 32, cs] = 1.0
        Wf[sk, cs] = 1.0
        Wb[a : a + 32, a : a + 32] = E.T
        Wb[sk, a : a + 32] = 1.0   # sink births β = 1 over all labels
        Wb[sk, sk] = 1.0
        Wb[a : a + 32, cs] = 1.0
        Wb[sk, cs] = 1.0
        Wbc[G + g, a : a + 32] = 1.0
        Wbc[G + g, sk] = 1.0
        Wcs[a : a + 32, g] = 1.0
        Wcs[sk, g] = 1.0
    bf = ml_dtypes.bfloat16
    return gp, lens, lg_cores, el32_cores, Wf.astype(bf), Wb.astype(bf), Wbc, Wcs.astype(bf)


def _log(msg):
    import time as _t

    print(f"[kernel {_t.strftime('%H:%M:%S')}] {msg}", flush=True)


def kernel(logits, trans, labels, seq_lens):
    global last_result
    from concourse.bass_utils import run_bass_kernel_spmd

    _log("host prep start")
    gp, lens, lg_cores, el32_cores, Wf, Wb, Wbc, Wcs = _host_prep(
        logits, trans, labels, seq_lens
    )
    _log("host prep done")

    if "nc" not in _prog_cache:
        _prog_cache["nc"] = _build_program()
        _log("program built")
    nc = _prog_cache["nc"]

    in_maps = [
        {
            "lg": lg_cores[i],
            "el32": el32_cores[i],
            "wf": Wf,
            "wbk": Wb,
            "wbc": Wbc,
            "wcs": Wcs,
        }
        for i in range(NCORES)
    ]
    r = run_bass_kernel_spmd(nc, in_maps, core_ids=list(range(NCORES)))
    last_result = r
    _log("device run done")

    # ---- unshard + select sink vs combine per sequence length ----
    devf = np.zeros(B, np.float32)
    devc = np.zeros(B, np.float32)
    for core in range(NCORES):
        rf = r.results[core]["resf"]
        rc = r.results[core]["resc"]
        b0 = core * BPC
        for g in range(G):
            ncols = NCOL if g < 2 else BPC - 2 * NCOL
            devf[b0 + g * NCOL : b0 + g * NCOL + ncols] = rf[g, :ncols]
            devc[b0 + g * NCOL : b0 + g * NCOL + ncols] = rc[g, :ncols]

    dev = np.where(lens <= F, devf, devc)
    logZ = dev + CSHIFT * lens.astype(np.float32)
    return (gp - logZ).astype(np.float32)



# revision 3
# speedup vs baseline: 2.8140x; 2.8140x over previous
"""CRF log-likelihood kernel for Trainium2 (Bass/Tile), 8-core data parallel.

out[b] = gold_path_score(b) - logZ(b)

logZ via chunked cold-start forward chains (exp domain).  The transfer
operator D_{el_t} Wf^T is strictly positive, so it contracts directions at
~0.3/step (Birkhoff): a chain started from the uniform vector forgets its
start after ~10 steps.  Split t = 1..T into C chunks of S steps; chain c
starts cold at t = c*S and runs BURN warm-up ticks + S valid ticks, all C
chains advancing in lockstep columns of the same matmul.  Serial depth drops
from T to SP = BURN + S ticks.

Scale recovery: chain c's state is gamma_c * alpha_t (unknown scalar
gamma_c).  At boundary time c*S + SP - 1 both chain c (final tick) and chain
c+1 (tick BURN) hold the same alpha direction, so the ratio of their Wf
column sums gives gamma_{c+1}/gamma_c exactly.  Sink rows capture
sum(alpha_{len-1}) at t == len per sequence (el32 gating), persist to the
chain end, and are read from the final tick's PSUM.  Host stitches:
logZ = log(sink_j) - log(gamma_j) + CSHIFT*len,  j = chunk containing len.

Per-core layout (128 sequences): partitions 0..95 = 3 label groups x 32,
96..98 = per-group sink rows; psum rows 99..101 = per-group column sums.
Columns: chain c occupies cols [43c, 43c+43); within a column, the 3 label
groups hold 3 different sequences (43+43+42+pad = 128).  Two column groups
of 10 chains each give two independent MM->TT streams that overlap on
PE/DVE.  Emissions exp'd on host, shipped bf16 (4x less DMA than f32
logits; el rounding is ~0.4% -> ~0.1 absolute logZ noise over 512 steps).
"""

import numpy as np
import ml_dtypes

B, T, L = 1024, 512, 32
NCORES = 8
BPC = B // NCORES        # 128 sequences per core
NCOL = 43                # columns per chain (3 label groups: 43+43+42+pad)
NACT = 96                # active label partitions
NPART = 99               # + 3 sink rows
MOUT = 102               # + 3 colsum rows
CSHIFT = 4.5
C = 20                   # chains (chunks)
BURN = 12                # cold-start warm-up ticks
S = (T - BURN) // C      # valid ticks per chain (25)
assert BURN + C * S == T
SP = BURN + S            # ticks per chain (37)
COLS = NCOL * C          # 860 total columns
NG = 2                   # column groups (independent MM->TT streams)
GCOLS = COLS // NG       # 430

_prog_cache = {}
last_result = None       # BassKernelResults of the most recent run (for test.py)


def _build_program():
    import concourse.bacc as bacc
    import concourse.tile as tile
    from concourse import mybir

    f32 = mybir.dt.float32
    bf16 = mybir.dt.bfloat16
    AF = mybir.ActivationFunctionType

    nc = bacc.Bacc("TRN2", target_bir_lowering=False, debug=False, num_devices=NCORES)
    el_d = nc.dram_tensor("el", [NPART, SP, COLS], bf16, kind="ExternalInput")
    u0_d = nc.dram_tensor("u0", [NPART, COLS], bf16, kind="ExternalInput")
    wf_d = nc.dram_tensor("wf", [NPART, MOUT], bf16, kind="ExternalInput")
    snapE_d = nc.dram_tensor("snapE", [6, COLS], f32, kind="ExternalOutput")
    snapF_d = nc.dram_tensor("snapF", [6, COLS], f32, kind="ExternalOutput")

    with tile.TileContext(nc) as tc:
        with (
            tc.tile_pool(name="consts", bufs=1) as consts,
            tc.tile_pool(name="elp", bufs=1) as elp,
            tc.tile_pool(name="uA", bufs=3) as uApool,
            tc.tile_pool(name="uB", bufs=3) as uBpool,
            tc.tile_pool(name="fin", bufs=1) as fin,
            tc.tile_pool(name="psA", bufs=2, space="PSUM") as psApool,
            tc.tile_pool(name="psB", bufs=2, space="PSUM") as psBpool,
        ):
            wf_sb = consts.tile([NPART, MOUT], bf16)
            u0_sb = consts.tile([NPART, COLS], bf16)
            el_sb = elp.tile([NPART, SP, COLS], bf16)
            finE = fin.tile([6, COLS], f32)
            finF = fin.tile([6, COLS], f32)

            nc.sync.dma_start(out=wf_sb[:], in_=wf_d[:])
            nc.sync.dma_start(out=u0_sb[:], in_=u0_d[:])
            # el streamed in tick-major chunks; first chunk small so the
            # chain starts as soon as possible.
            edges = [0, 2, 6, 10, 14, 18, 22, 26, 30, 34, SP]
            for k0, k1 in zip(edges[:-1], edges[1:]):
                nc.sync.dma_start(
                    out=el_sb[:, k0:k1, :], in_=el_d[:, k0:k1, :]
                )

            groups = [
                (uApool, psApool, 0, GCOLS),
                (uBpool, psBpool, GCOLS, COLS),
            ]
            uprev = [u0_sb[:, c0:c1] for (_, _, c0, c1) in groups]
            for k in range(1, SP + 1):
                for gi, (upool, pspool, c0, c1) in enumerate(groups):
                    ps = pspool.tile([MOUT, GCOLS], f32, tag=f"ps{gi}")
                    nc.tensor.matmul(ps[:], wf_sb[:], uprev[gi], start=True, stop=True)
                    if k < SP:
                        un = upool.tile([NPART, GCOLS], bf16, tag=f"u{gi}")
                        nc.vector.tensor_mul(
                            un[:], ps[0:NPART, :], el_sb[:, k - 1, c0:c1]
                        )
                        uprev[gi] = un[:]
                    if k == BURN:
                        nc.scalar.activation(
                            finE[:, c0:c1], ps[NACT:MOUT, :], AF.Copy
                        )
                    if k == SP:
                        nc.scalar.activation(
                            finF[:, c0:c1], ps[NACT:MOUT, :], AF.Copy
                        )
            nc.sync.dma_start(out=snapE_d[:], in_=finE[:])
            nc.sync.dma_start(out=snapF_d[:], in_=finF[:])

    nc.compile()
    return nc


def _host_prep(logits, trans, labels, seq_lens):
    logits = np.ascontiguousarray(np.asarray(logits), dtype=np.float32)
    trans = np.asarray(trans, dtype=np.float32)
    labels = np.asarray(labels)
    lens = np.clip(np.asarray(seq_lens), 1, T).astype(np.int64)

    # ---- gold path score (host: index gathers over small inputs) ----
    tmask = np.arange(T)[None, :] < lens[:, None]
    unary = np.take_along_axis(logits, labels[..., None].astype(np.int64), axis=2)[..., 0]
    gp = (unary * tmask).sum(1) + (trans[labels[:, :-1], labels[:, 1:]] * tmask[:, 1:]).sum(1)

    # ---- emissions: exp on host, masked past seq end; slice t=T is
    # capture-only (el=0 everywhere, el32=1) ----
    lgx = logits.copy()
    lgx[~tmask] = -np.inf
    el_full = np.exp(lgx - CSHIFT)                                   # [B,T,L]
    el_full = np.concatenate([el_full, np.zeros((B, 1, L), np.float32)], axis=1)
    el32 = (np.arange(T + 1)[None, :] >= lens[:, None]).astype(np.float32)  # [B,T+1]

    bf = ml_dtypes.bfloat16
    gsl = [(0, 43), (43, 86), (86, 128)]  # local seq ranges per label group
    el_cores, u0_cores = [], []
    for core in range(NCORES):
        b0 = core * BPC
        E = el_full[b0 : b0 + BPC]          # [128, T+1, L]
        E32 = el32[b0 : b0 + BPC]           # [128, T+1]
        packed = np.zeros((NPART, SP, COLS), np.float32)
        u0 = np.zeros((NPART, COLS), np.float32)
        for c in range(C):
            t0 = c * S
            sl = E[:, t0 + 1 : t0 + SP + 1, :]    # [128, SP, L]
            sl32 = E32[:, t0 + 1 : t0 + SP + 1]   # [128, SP]
            for g, (s0, s1) in enumerate(gsl):
                nc_ = s1 - s0
                cc = NCOL * c
                packed[32 * g : 32 * g + 32, :, cc : cc + nc_] = sl[s0:s1].transpose(2, 1, 0)
                packed[NACT + g, :, cc : cc + nc_] = sl32[s0:s1].T
                if c == 0:
                    u0[32 * g : 32 * g + 32, cc : cc + nc_] = E[s0:s1, 0, :].T
                else:
                    u0[32 * g : 32 * g + 32, cc : cc + nc_] = 1.0
        el_cores.append(packed.astype(bf))
        u0_cores.append(u0.astype(bf))

    # ---- stationary operator: block-diag exp(trans) + sink + colsum ----
    Ew = np.exp(trans).astype(np.float32)
    Wf = np.zeros((NPART, MOUT), np.float32)
    for g in range(3):
        a, sk, cs = 32 * g, NACT + g, NPART + g
        Wf[a : a + 32, a : a + 32] = Ew
        Wf[a : a + 32, sk] = 1.0
        Wf[sk, sk] = 1.0
        Wf[a : a + 32, cs] = 1.0
        Wf[sk, cs] = 1.0
    return gp, lens, el_cores, u0_cores, Wf.astype(bf)


def _log(msg):
    import time as _t

    print(f"[kernel {_t.strftime('%H:%M:%S')}] {msg}", flush=True)


def kernel(logits, trans, labels, seq_lens):
    global last_result
    from concourse.bass_utils import run_bass_kernel_spmd

    _log("host prep start")
    gp, lens, el_cores, u0_cores, Wf = _host_prep(logits, trans, labels, seq_lens)
    _log("host prep done")

    if "nc" not in _prog_cache:
        _prog_cache["nc"] = _build_program()
        _log("program built")
    nc = _prog_cache["nc"]

    in_maps = [
        {"el": el_cores[i], "u0": u0_cores[i], "wf": Wf}
        for i in range(NCORES)
    ]
    r = run_bass_kernel_spmd(nc, in_maps, core_ids=list(range(NCORES)))
    last_result = r
    _log("device run done")

    # ---- unshard: per-core [3,COLS]/[6,COLS] -> per-sequence chain arrays ----
    gsl = [(0, 43), (43, 86), (86, 128)]
    colE = np.zeros((C, B), np.float64)   # chain colsum at its tick BURN
    colF = np.zeros((C, B), np.float64)   # chain colsum at its final tick
    sinkF = np.zeros((C, B), np.float64)  # chain sink at its final tick
    for core in range(NCORES):
        sE = np.asarray(last_result.results[core]["snapE"], np.float64)  # [6,COLS]
        sF = np.asarray(last_result.results[core]["snapF"], np.float64)  # [6,COLS]
        b0 = core * BPC
        for g, (s0, s1) in enumerate(gsl):
            nc_ = s1 - s0
            colE[:, b0 + s0 : b0 + s1] = sE[3 + g].reshape(C, NCOL)[:, :nc_]
            sinkF[:, b0 + s0 : b0 + s1] = sF[g].reshape(C, NCOL)[:, :nc_]
            colF[:, b0 + s0 : b0 + s1] = sF[3 + g].reshape(C, NCOL)[:, :nc_]

    # ---- stitch scales: chain c valid for len in (c*S+BURN, c*S+SP] ----
    j = np.zeros(B, np.int64)
    for c in range(1, C):
        j[lens > c * S + BURN] = c
    with np.errstate(divide="ignore", invalid="ignore"):
        log_rho = np.log(colE[1:]) - np.log(colF[:-1])        # [C-1, B]
        log_gamma = np.concatenate(
            [np.zeros((1, B)), np.cumsum(log_rho, axis=0)], axis=0
        )                                                      # [C, B]
        log_sink = np.log(sinkF[j, np.arange(B)])
    logZ = log_sink - log_gamma[j, np.arange(B)] + CSHIFT * lens
    return (gp - logZ).astype(np.float32)
